# revision 10
# baseline (speedup 1.0000x reference)
"""BinaryMaskBilateralFilter TRN2 kernel.

Input x: (8, 8, 512, 512) f32 in [0,1]. Shard batch dim across 8 NeuronCores
(1 example = 8 channels of 512x512 per core). Per iteration (2 total), the
7x7 gaussian blur of mask and mask^2 is computed as 7 PSUM-accumulated fp32
band matmuls per 122-row output window: the stationary operand is an H-band
matrix holding column delta_w of the 2D gaussian; the moving operand is the
w-padded image tile shifted by delta_w in the free dim. The bilateral combine
runs on DVE/ACT. Iterations round-trip through internal DRAM.
"""
import numpy as np

import concourse.bacc as bacc
import concourse.mybir as mybir
from concourse import tile
from concourse import bass_utils

F32 = mybir.dt.float32
AF = mybir.ActivationFunctionType
ALU = mybir.AluOpType

B, C, H, W = 8, 8, 512, 512
K = 7
PAD = 3
WPAD = W + 2 * PAD  # 518
NUM_ITERS = 2
THRESHOLD = 0.5

# h windows: (row_start, K_rows, out_start, M_out, center_part_offset, band)
WINDOWS = [
    (0, 125, 0, 122, 0, "A"),
    (119, 128, 122, 122, 3, "B"),
    (241, 128, 244, 122, 3, "B"),
    (363, 128, 366, 122, 3, "B"),
    (485, 27, 488, 24, 3, "B"),
]
MB = 122  # band column block


def _gauss2d():
    c = np.arange(K, dtype=np.float64) - (K - 1) / 2.0
    g = np.exp(-(c[:, None] ** 2 + c[None, :] ** 2) / (2.0 * 1.5 ** 2))
    return g / g.sum()  # [dh, dw] float64


def make_bands():
    g = _gauss2d()
    bandsA = np.zeros((128, K * MB), np.float32)
    bandsB = np.zeros((128, K * MB), np.float32)
    for dw in range(K):
        for m in range(MB):
            for dh in range(K):
                # A: B[k, m] = g2d[k - m + 3, dw]  -> k = m + dh - 3
                k = m + dh - 3
                if 0 <= k < 128:
                    bandsA[k, dw * MB + m] = np.float32(g[dh, dw])
                # B: B[k, m] = g2d[k - m, dw]      -> k = m + dh
                k = m + dh
                if 0 <= k < 128:
                    bandsB[k, dw * MB + m] = np.float32(g[dh, dw])
    return bandsA, bandsB


def _emit(nc, tc, pools, x, bandsA, bandsB, y, maskbuf):
    bands_pool, mpool, m2pool, ps, tmp = pools
    bA = bands_pool.tile([128, K * MB], F32, name="bA")
    bB = bands_pool.tile([128, K * MB], F32, name="bB")
    nc.sync.dma_start(bA[:, :], bandsA[:, :])
    nc.sync.dma_start(bB[:, :], bandsB[:, :])

    for it in range(NUM_ITERS):
        src = x if it == 0 else maskbuf
        for ch in range(C):
            for (s, kk, o, m, p0, bname) in WINDOWS:
                bt = bA if bname == "A" else bB
                mt = mpool.tile([128, WPAD], F32, name=f"mt_{it}_{ch}_{o}",
                                tag="mt")
                nc.vector.memset(mt[:, 0:PAD], 0.0)
                nc.vector.memset(mt[:, W + PAD:WPAD], 0.0)
                nc.sync.dma_start(mt[0:kk, PAD:W + PAD], src[ch, s:s + kk, :])
                m2t = m2pool.tile([128, WPAD], F32, name=f"m2t_{it}_{ch}_{o}",
                                  tag="m2t")
                nc.scalar.activation(m2t[0:kk, :], mt[0:kk, :], AF.Square)

                psf = ps.tile([128, W], F32, name=f"psf_{it}_{ch}_{o}",
                              tag="psf")
                psm = ps.tile([128, W], F32, name=f"psm_{it}_{ch}_{o}",
                              tag="psm")
                # symmetry-folded shifts: g2d[:, 3+e] == g2d[:, 3-e], so
                # pair-sum the +-e shifted slices once (GPSIMD for mask,
                # DVE for mask^2) and run 4 matmul streams instead of 7.
                fsrcs = [(3, mt[0:kk, PAD:PAD + W])]
                msrcs = [(3, m2t[0:kk, PAD:PAD + W])]
                for e in (1, 2, 3):
                    se = mpool.tile([128, W], F32,
                                    name=f"se{e}_{it}_{ch}_{o}", tag=f"se{e}")
                    nc.gpsimd.tensor_tensor(
                        se[0:kk, :], mt[0:kk, PAD + e:PAD + e + W],
                        mt[0:kk, PAD - e:PAD - e + W], op=ALU.add)
                    sq = m2pool.tile([128, W], F32,
                                     name=f"sq{e}_{it}_{ch}_{o}", tag=f"sq{e}")
                    nc.vector.tensor_tensor(
                        sq[0:kk, :], m2t[0:kk, PAD + e:PAD + e + W],
                        m2t[0:kk, PAD - e:PAD - e + W], op=ALU.add)
                    fsrcs.append((3 - e, se[0:kk, :]))
                    msrcs.append((3 - e, sq[0:kk, :]))
                # col-tiled matmuls: 4 concurrent 32-row output groups
                if m > 32:
                    groups = [(mo, min(32, m - mo)) for mo in range(0, m, 32)]
                else:
                    groups = [(0, m)]
                for psum, srcs in ((psf, fsrcs), (psm, msrcs)):
                    for si, (dw, rhs) in enumerate(srcs):
                        for (mo, mw) in groups:
                            nc.tensor.matmul(
                                psum[mo:mo + mw, :],
                                bt[0:kk, dw * MB + mo:dw * MB + mo + mw],
                                rhs,
                                start=(si == 0), stop=(si == len(srcs) - 1),
                                tile_position=(0, mo),
                                skip_group_check=True)

                mct = mpool.tile([128, W], F32, name=f"mct_{it}_{ch}_{o}",
                                 tag="mct")
                nc.sync.dma_start(mct[0:m, :], src[ch, o:o + m, :])
                mc = mct[0:m, :]
                f2 = tmp.tile([128, W], F32, name=f"f2_{it}_{ch}_{o}", tag="f2")
                nc.scalar.activation(f2[0:m, :], psf[0:m, :], AF.Square)
                q = tmp.tile([128, W], F32, name=f"q_{it}_{ch}_{o}", tag="q")
                nc.vector.scalar_tensor_tensor(
                    q[0:m, :], f2[0:m, :], -1.0, psm[0:m, :], ALU.mult, ALU.add)
                v = tmp.tile([128, W], F32, name=f"v_{it}_{ch}_{o}", tag="v")
                nc.vector.tensor_scalar(v[0:m, :], q[0:m, :], 0.0, -10.0,
                                        ALU.max, ALU.mult)
                ew = tmp.tile([128, W], F32, name=f"ew_{it}_{ch}_{o}", tag="ew")
                nc.scalar.activation(ew[0:m, :], v[0:m, :], AF.Exp)
                d = tmp.tile([128, W], F32, name=f"d_{it}_{ch}_{o}", tag="d")
                nc.vector.scalar_tensor_tensor(
                    d[0:m, :], mc, -1.0, psf[0:m, :], ALU.mult, ALU.add)
                p = tmp.tile([128, W], F32, name=f"p_{it}_{ch}_{o}", tag="p")
                nc.vector.tensor_tensor(p[0:m, :], ew[0:m, :], d[0:m, :],
                                        op=ALU.mult)
                mn = tmp.tile([128, W], F32, name=f"mn_{it}_{ch}_{o}", tag="mn")
                nc.vector.tensor_tensor(mn[0:m, :], mc, p[0:m, :], op=ALU.add)
                if it < NUM_ITERS - 1:
                    nc.sync.dma_start(maskbuf[ch, o:o + m, :], mn[0:m, :])
                else:
                    thr = tmp.tile([128, W], F32, name=f"thr_{ch}_{o}",
                                   tag="thr")
                    nc.vector.tensor_scalar(thr[0:m, :], mn[0:m, :],
                                            THRESHOLD, None, ALU.is_gt)
                    nc.sync.dma_start(y[ch, o:o + m, :], thr[0:m, :])


def build_program():
    nc = bacc.Bacc(trn_type="TRN2", target_bir_lowering=False, debug=False,
                   num_devices=8)
    x = nc.dram_tensor("x", [C, H, W], F32, kind="ExternalInput").ap()
    bandsA = nc.dram_tensor("bandsA", [128, K * MB], F32,
                            kind="ExternalInput").ap()
    bandsB = nc.dram_tensor("bandsB", [128, K * MB], F32,
                            kind="ExternalInput").ap()
    y = nc.dram_tensor("y", [C, H, W], F32, kind="ExternalOutput").ap()
    maskbuf = nc.dram_tensor("maskbuf", [C, H, W], F32, kind="Internal").ap()

    with tile.TileContext(nc) as tc:
        with (
            tc.tile_pool(name="bands", bufs=1) as bands_pool,
            tc.tile_pool(name="mtiles", bufs=4) as mpool,
            tc.tile_pool(name="m2tiles", bufs=3) as m2pool,
            tc.tile_pool(name="ps", bufs=4, space="PSUM") as ps,
            tc.tile_pool(name="tmp", bufs=4) as tmp,
        ):
            _emit(nc, tc, (bands_pool, mpool, m2pool, ps, tmp),
                  x, bandsA, bandsB, y, maskbuf)
    nc.compile()
    return nc


_cached = {}


def _make_runner(nc):
    """Build a cached 8-core shard_map runner for the compiled program."""
    import jax
    from jax.sharding import Mesh, PartitionSpec
    from jax.experimental.shard_map import shard_map
    from concourse import bass2jax

    bass2jax.install_neuronx_cc_hook()
    partition_name = (nc.partition_id_tensor.name
                      if nc.partition_id_tensor else None)
    in_names, out_names, out_avals = [], [], []
    for alloc in nc.m.functions[0].allocations:
        if not isinstance(alloc, mybir.MemoryLocationSet):
            continue
        name = alloc.memorylocations[0].name
        if alloc.kind == "ExternalInput":
            if name != partition_name:
                in_names.append(name)
        elif alloc.kind == "ExternalOutput":
            out_names.append(name)
            out_avals.append(jax.core.ShapedArray(
                tuple(alloc.tensor_shape), mybir.dt.np(alloc.dtype)))
    n_params = len(in_names)
    all_names = list(in_names) + list(out_names)
    if partition_name is not None:
        all_names.append(partition_name)
    out_shapes = [(a.shape, a.dtype) for a in out_avals]

    def _body(*args):
        operands = list(args)
        if partition_name is not None:
            operands.append(bass2jax.partition_id_tensor())
        outs = bass2jax._bass_exec_p.bind(
            *operands, out_avals=tuple(out_avals), in_names=tuple(all_names),
            out_names=tuple(out_names), lowering_input_output_aliases=(),
            sim_require_finite=True, sim_require_nnan=True, nc=nc)
        return tuple(outs)

    try:
        devices = jax.devices("axon")[:B]
    except RuntimeError:
        devices = jax.devices()[:B]
    assert len(devices) == B, f"need {B} neuron cores, have {len(devices)}"
    mesh = Mesh(np.asarray(devices), ("core",))
    n_outs = len(out_names)
    sharded = jax.jit(
        shard_map(_body, mesh=mesh,
                  in_specs=(PartitionSpec("core"),) * (n_params + n_outs),
                  out_specs=(PartitionSpec("core"),) * n_outs,
                  check_rep=False),
        donate_argnums=tuple(range(n_params, n_params + n_outs)),
        keep_unused=True)

    def run(in_maps):
        concat_in = [
            np.concatenate([np.asarray(m[n]) for m in in_maps], axis=0)
            for n in in_names
        ]
        zeros = [np.zeros((B * s[0], *s[1:]), d) for (s, d) in out_shapes]
        outs = sharded(*concat_in, *zeros)
        return {
            name: np.asarray(outs[i]).reshape(B, *out_shapes[i][0])
            for i, name in enumerate(out_names)
        }

    return run


def kernel(x: np.ndarray) -> np.ndarray:
    x = np.ascontiguousarray(np.asarray(x, dtype=np.float32))
    assert x.shape == (B, C, H, W)
    if "run" not in _cached:
        nc = build_program()
        _cached["bands"] = make_bands()
        try:
            _cached["run"] = _make_runner(nc)
        except Exception:
            _cached["nc"] = nc
            _cached["run"] = None
    bandsA, bandsB = _cached["bands"]
    in_maps = [
        {"x": x[i], "bandsA": bandsA, "bandsB": bandsB}
        for i in range(B)
    ]
    if _cached["run"] is not None:
        outs = _cached["run"](in_maps)
        return np.ascontiguousarray(outs["y"])
    res = bass_utils.run_bass_kernel_spmd(
        _cached["nc"], in_maps, core_ids=list(range(B)))
    return np.stack([res.results[i]["y"] for i in range(B)], axis=0)
